# revision 22
# baseline (speedup 1.0000x reference)
"""CRF tagger NLL loss kernel for Trainium2 (8 NeuronCores, data-parallel over batch).

Math (matches torchcrf-style reference with mask == all-ones):
  em = Z @ W.T                               (bias folded in on host)
  numerator_b = start[t0] + sum_l em[l, t_l] + sum_l bias[t_l]
                + sum_l trans[t_l, t_{l+1}] + end[t_last]
  log_z_b via forward algorithm over L=2048 steps, C=5 states.

This problem is memory-bound on streaming Z (16 MB f32 per core). Device
work per core (B_loc=4 batches):
  * stream Z^T in fp8(e4m3) from HBM (host pre-transposed/quantized; W
    scaled by 16 to keep fp8 weights out of the subnormal range), PE
    DoubleRow matmul em'^T = (16 W) @ Z^T (256-deep contraction/pass)
  * drain PSUM -> SBUF (split between ACT and DVE), em' -> HBM
The first batch's Z loads are split so the PE can start early; the PE
then runs an uninterrupted DMA-paced matmul stream.
Host divides by 16 and runs the O(B*L*C^2) forward recurrence + gold
path score in float64 (0.5% of the FLOPs; numerically exact).
"""

import sys

import numpy as np

for _p in ("/opt/trn_rl_repo", "/opt/pypackages"):
    if _p not in sys.path:
        sys.path.append(_p)

B, L, D, C = 32, 2048, 512, 5
N_CORES = 8
B_LOC = B // N_CORES  # 4
WSCALE = 16.0  # W premultiplier to keep fp8 weights out of subnormal range
KBP = 2  # 256-deep contraction blocks (DoubleRow)
LC = 512  # psum free-dim chunk (one bank)
NLC = L // LC
DTYPE_MODE = "fp8dr"  # informational; single build path

_cache = {}


def _build():
    import concourse.bacc as bacc
    import concourse.mybir as mybir
    import concourse.tile as tile
    from concourse.bass import ts

    f32 = mybir.dt.float32
    fp8 = mybir.dt.float8e4

    nc = bacc.Bacc("TRN2", target_bir_lowering=False, debug=False)

    zt_d = nc.dram_tensor("zt", [B_LOC, KBP, 128, 2, L], fp8, kind="ExternalInput")
    wt_d = nc.dram_tensor("wt", [KBP, 128, 2, 16], fp8, kind="ExternalInput")
    bf16 = mybir.dt.bfloat16
    em_d = nc.dram_tensor("em_out", [B_LOC, C, L], bf16, kind="ExternalOutput")

    with tile.TileContext(nc) as tc:
        with (
            tc.tile_pool(name="const", bufs=1) as cpool,
            tc.tile_pool(name="zpool", bufs=8) as zpool,
            tc.tile_pool(name="empool", bufs=4) as empool,
            tc.tile_pool(name="pspool", bufs=4, space="PSUM") as ppool,
        ):
            wt_sb = cpool.tile([128, KBP, 2, 16], fp8)
            nc.scalar.dma_start(
                out=wt_sb[:], in_=wt_d.ap().rearrange("kb p i c -> p kb i c")
            )

            # All z loads on ONE queue (sync), whole 512KB tiles, hoisted
            # ahead of all compute. A single issuing queue gives FIFO
            # completion order (multi-queue DMAs round-robin at packet
            # granularity and all finish late together), so each batch's
            # data lands in sequence and compute chases the stream.
            all_z = []
            for b in range(B_LOC):
                z_tiles = []
                for kbp in range(KBP):
                    z_sb = zpool.tile(
                        [128, 2, L], fp8, tag=f"z{kbp}", name=f"z_{b}_{kbp}"
                    )
                    if (b == 0 and kbp == 0) or (
                        b == B_LOC - 1 and kbp == KBP - 1
                    ):
                        # first and last tiles in halves (same queue keeps
                        # FIFO): the first matmuls start ~1us earlier, and
                        # the last batch's kbp1 matmuls chase the stream
                        # instead of waiting for the full final tile
                        half = L // 2
                        for hh in range(2):
                            nc.sync.dma_start(
                                out=z_sb[:, :, ts(hh, half)],
                                in_=zt_d[b, kbp, :, :, ts(hh, half)],
                            )
                    else:
                        nc.sync.dma_start(out=z_sb[:], in_=zt_d[b, kbp])
                    z_tiles.append(z_sb)
                all_z.append(z_tiles)

            for b in range(B_LOC):
                z_tiles = all_z[b]
                # two psum tiles of 2 banks each per batch
                psums = [
                    ppool.tile([C, 2 * LC], f32, tag="em_ps", name=f"ps_{b}_{i}")
                    for i in range(2)
                ]
                for kbp in range(KBP):
                    for lc in range(NLC):
                        nc.tensor.matmul(
                            psums[lc // 2][:, ts(lc % 2, LC)],
                            lhsT=wt_sb[:, kbp, :, 0:C],
                            rhs=z_tiles[kbp][:, :, ts(lc, LC)],
                            start=(kbp == 0),
                            stop=(kbp == KBP - 1),
                            perf_mode=mybir.MatmulPerfMode.DoubleRow,
                        )
                em_sb = empool.tile([C, L], bf16, tag="em", name=f"em_sb_{b}")
                # drain psum chunks in parallel on ACT and DVE; write em to
                # HBM as soon as chunks land (the last batch per-chunk on
                # two queues to shorten the tail)
                last = b == B_LOC - 1
                for half_i in range(2):
                    p = psums[half_i]
                    nc.scalar.copy(
                        em_sb[:, ts(2 * half_i, LC)], p[:, ts(0, LC)]
                    )
                    nc.vector.tensor_copy(
                        out=em_sb[:, ts(2 * half_i + 1, LC)], in_=p[:, ts(1, LC)]
                    )
                    if last:
                        for ci in range(2):
                            lc = 2 * half_i + ci
                            q = nc.scalar if ci == 0 else nc.gpsimd
                            q.dma_start(
                                out=em_d[b, :, ts(lc, LC)],
                                in_=em_sb[:, ts(lc, LC)],
                            )
                    else:
                        nc.gpsimd.dma_start(
                            out=em_d[b, :, ts(half_i, 2 * LC)],
                            in_=em_sb[:, ts(half_i, 2 * LC)],
                        )

    nc.compile()
    return nc


def _get_nc(dtype_mode=None):
    if "k" not in _cache:
        _cache["k"] = _build()
    return _cache["k"]


def _host_prep(Z, W, bias_c, transitions, dtype_mode=None):
    """Build per-core input maps."""
    import ml_dtypes

    fp8 = ml_dtypes.float8_e4m3

    # wt[kbp, p, i, 0:C] = WSCALE * W[c, kbp*256 + i*128 + p]; c-dim padded
    # to 16 (dual-fp8 ldweights needs 16-elem-aligned outer AP steps)
    WT = np.ascontiguousarray(W.T) * WSCALE  # [D, C]
    wt5 = WT.reshape(KBP, 2, 128, C).transpose(0, 2, 1, 3)  # [KBP,128,2,C]
    wt = np.zeros((KBP, 128, 2, 16), np.float32)
    wt[..., :C] = wt5
    wt = wt.astype(fp8)

    in_maps = []
    for ci in range(N_CORES):
        Zc = Z[ci * B_LOC : (ci + 1) * B_LOC]  # [B_LOC, L, D]
        zt = Zc.transpose(0, 2, 1)  # [B_LOC, D, L]
        # zt[b, kbp, p, i, l] = Z[b, l, kbp*256 + i*128 + p]
        ztp = np.ascontiguousarray(
            zt.reshape(B_LOC, KBP, 2, 128, L).transpose(0, 1, 3, 2, 4)
        ).astype(fp8)
        in_maps.append({"zt": ztp, "wt": wt})
    return in_maps


def _host_finish(results, tags, start_t, end_t, bias_c, transitions):
    """Combine per-core em outputs into the scalar loss (float64 host math)."""
    st = start_t.astype(np.float64)
    en = end_t.astype(np.float64)
    cb = bias_c.astype(np.float64)
    tr = transitions.astype(np.float64)

    em_all = np.concatenate(
        [np.asarray(results[ci]["em_out"], dtype=np.float64)
         for ci in range(N_CORES)], axis=0
    ) / WSCALE  # [B, C, L]

    tags = tags.astype(np.int64)
    l_idx = np.arange(L)
    b_idx = np.arange(B)[:, None]

    # numerator
    em_tag_sum = em_all[b_idx, tags, l_idx[None, :]].sum(axis=1)  # [B]
    bias_sum = cb[tags].sum(axis=1)
    trans_sum = tr[tags[:, :-1], tags[:, 1:]].sum(axis=1)
    numerator = st[tags[:, 0]] + en[tags[:, -1]] + em_tag_sum + bias_sum + trans_sum

    # log_z: forward recurrence in probability space with renormalization.
    # alpha_{t} = (alpha_{t-1} @ EB) * exp(em_t - m_t)
    EB = np.exp(tr + cb[None, :])  # [C, C] includes per-step bias
    alpha0 = st + cb + em_all[:, :, 0].copy()  # [B, C] log space
    m0 = alpha0.max(axis=1)
    v = np.exp(alpha0 - m0[:, None])
    log_z = m0.copy()
    for t in range(1, L):
        e_t = em_all[:, :, t]  # [B, C]
        m_t = e_t.max(axis=1)
        v = (v @ EB) * np.exp(e_t - m_t[:, None])
        s = v.max(axis=1)
        v /= s[:, None]
        log_z += np.log(s) + m_t
    log_z += np.log((v * np.exp(en)[None, :]).sum(axis=1))

    return np.float32(np.mean(log_z - numerator))


def kernel(**inputs):
    from concourse.bass_utils import run_bass_kernel_spmd

    Z = np.asarray(inputs["Z"], dtype=np.float32)
    tags = np.asarray(inputs["tags"])
    W = np.asarray(inputs["W"], dtype=np.float32)
    b_ = np.asarray(inputs["b"], dtype=np.float32)
    cb = np.asarray(inputs["class_bias"], dtype=np.float32)
    st = np.asarray(inputs["start_trans"], dtype=np.float32)
    en = np.asarray(inputs["end_trans"], dtype=np.float32)
    tr = np.asarray(inputs["transitions"], dtype=np.float32)

    bias_c = b_ + cb
    nc = _get_nc()
    in_maps = _host_prep(Z, W, bias_c, tr)
    res = run_bass_kernel_spmd(nc, in_maps, core_ids=list(range(N_CORES)))
    return _host_finish(res.results, tags, st, en, bias_c, tr)


# revision 23
# speedup vs baseline: 1.1295x; 1.1295x over previous
"""CRF tagger NLL loss kernel for Trainium2 (8 NeuronCores, data-parallel over batch).

Math (matches torchcrf-style reference with mask == all-ones):
  em = Z @ W.T                               (bias folded in on host)
  numerator_b = start[t0] + sum_l em[l, t_l] + sum_l bias[t_l]
                + sum_l trans[t_l, t_{l+1}] + end[t_last]
  log_z_b via forward algorithm over L=2048 steps, C=5 states.

This problem is memory-bound on streaming Z (16 MB f32 per core). Device
work per core (B_loc=4 batches):
  * stream Z^T in fp8(e4m3) from HBM (host pre-transposed/quantized; W
    scaled by 16 to keep fp8 weights out of the subnormal range), PE
    DoubleRow matmul em'^T = (16 W) @ Z^T (256-deep contraction/pass)
  * drain PSUM -> SBUF (split between ACT and DVE), em' -> HBM
The first batch's Z loads are split so the PE can start early; the PE
then runs an uninterrupted DMA-paced matmul stream.
Host divides by 16 and runs the O(B*L*C^2) forward recurrence + gold
path score in float64 (0.5% of the FLOPs; numerically exact).
"""

import sys

import numpy as np

for _p in ("/opt/trn_rl_repo", "/opt/pypackages"):
    if _p not in sys.path:
        sys.path.append(_p)

B, L, D, C = 32, 2048, 512, 5
N_CORES = 8
B_LOC = B // N_CORES  # 4
WSCALE = 16.0  # W premultiplier to keep fp8 weights out of subnormal range
KBP = 2  # 256-deep contraction blocks (DoubleRow)
LC = 512  # psum free-dim chunk (one bank)
NLC = L // LC
DTYPE_MODE = "fp8dr"  # informational; single build path

_cache = {}


def _build():
    import concourse.bacc as bacc
    import concourse.mybir as mybir
    import concourse.tile as tile
    from concourse.bass import ts

    f32 = mybir.dt.float32
    fp8 = mybir.dt.float8e4

    nc = bacc.Bacc("TRN2", target_bir_lowering=False, debug=False)

    zt_d = nc.dram_tensor("zt", [B_LOC, KBP, 128, 2, L], fp8, kind="ExternalInput")
    wt_d = nc.dram_tensor("wt", [KBP, 128, 2, 16], fp8, kind="ExternalInput")
    bf16 = mybir.dt.bfloat16
    em_d = nc.dram_tensor("em_out", [B_LOC, C, L], bf16, kind="ExternalOutput")

    with tile.TileContext(nc) as tc:
        with (
            tc.tile_pool(name="const", bufs=1) as cpool,
            tc.tile_pool(name="zpool", bufs=8) as zpool,
            tc.tile_pool(name="empool", bufs=4) as empool,
            tc.tile_pool(name="pspool", bufs=4, space="PSUM") as ppool,
        ):
            wt_sb = cpool.tile([128, KBP, 2, 16], fp8)
            nc.scalar.dma_start(
                out=wt_sb[:], in_=wt_d.ap().rearrange("kb p i c -> p kb i c")
            )

            # All z loads on ONE queue (sync), whole 512KB tiles, hoisted
            # ahead of all compute. A single issuing queue gives FIFO
            # completion order (multi-queue DMAs round-robin at packet
            # granularity and all finish late together), so each batch's
            # data lands in sequence and compute chases the stream.
            all_z = []
            for b in range(B_LOC):
                z_tiles = []
                for kbp in range(KBP):
                    z_sb = zpool.tile(
                        [128, 2, L], fp8, tag=f"z{kbp}", name=f"z_{b}_{kbp}"
                    )
                    if b == 0 and kbp == 0:
                        # first tile in halves (same queue keeps FIFO) so
                        # the first matmuls start ~1us earlier
                        half = L // 2
                        for hh in range(2):
                            nc.sync.dma_start(
                                out=z_sb[:, :, ts(hh, half)],
                                in_=zt_d[b, kbp, :, :, ts(hh, half)],
                            )
                    else:
                        nc.sync.dma_start(out=z_sb[:], in_=zt_d[b, kbp])
                    z_tiles.append(z_sb)
                all_z.append(z_tiles)

            for b in range(B_LOC):
                z_tiles = all_z[b]
                # two psum tiles of 2 banks each per batch
                psums = [
                    ppool.tile([C, 2 * LC], f32, tag="em_ps", name=f"ps_{b}_{i}")
                    for i in range(2)
                ]
                for kbp in range(KBP):
                    for lc in range(NLC):
                        nc.tensor.matmul(
                            psums[lc // 2][:, ts(lc % 2, LC)],
                            lhsT=wt_sb[:, kbp, :, 0:C],
                            rhs=z_tiles[kbp][:, :, ts(lc, LC)],
                            start=(kbp == 0),
                            stop=(kbp == KBP - 1),
                            perf_mode=mybir.MatmulPerfMode.DoubleRow,
                        )
                em_sb = empool.tile([C, L], bf16, tag="em", name=f"em_sb_{b}")
                # drain psum chunks in parallel on ACT and DVE; write em to
                # HBM as soon as chunks land (the last batch per-chunk on
                # two queues to shorten the tail)
                last = b == B_LOC - 1
                for half_i in range(2):
                    p = psums[half_i]
                    nc.scalar.copy(
                        em_sb[:, ts(2 * half_i, LC)], p[:, ts(0, LC)]
                    )
                    nc.vector.tensor_copy(
                        out=em_sb[:, ts(2 * half_i + 1, LC)], in_=p[:, ts(1, LC)]
                    )
                    if last:
                        for ci in range(2):
                            lc = 2 * half_i + ci
                            q = nc.scalar if ci == 0 else nc.gpsimd
                            q.dma_start(
                                out=em_d[b, :, ts(lc, LC)],
                                in_=em_sb[:, ts(lc, LC)],
                            )
                    else:
                        nc.gpsimd.dma_start(
                            out=em_d[b, :, ts(half_i, 2 * LC)],
                            in_=em_sb[:, ts(half_i, 2 * LC)],
                        )

    nc.compile()
    return nc


def _get_nc(dtype_mode=None):
    if "k" not in _cache:
        _cache["k"] = _build()
    return _cache["k"]


def _host_prep(Z, W, bias_c, transitions, dtype_mode=None):
    """Build per-core input maps."""
    import ml_dtypes

    fp8 = ml_dtypes.float8_e4m3

    # wt[kbp, p, i, 0:C] = WSCALE * W[c, kbp*256 + i*128 + p]; c-dim padded
    # to 16 (dual-fp8 ldweights needs 16-elem-aligned outer AP steps)
    WT = np.ascontiguousarray(W.T) * WSCALE  # [D, C]
    wt5 = WT.reshape(KBP, 2, 128, C).transpose(0, 2, 1, 3)  # [KBP,128,2,C]
    wt = np.zeros((KBP, 128, 2, 16), np.float32)
    wt[..., :C] = wt5
    wt = wt.astype(fp8)

    in_maps = []
    for ci in range(N_CORES):
        Zc = Z[ci * B_LOC : (ci + 1) * B_LOC]  # [B_LOC, L, D]
        zt = Zc.transpose(0, 2, 1)  # [B_LOC, D, L]
        # zt[b, kbp, p, i, l] = Z[b, l, kbp*256 + i*128 + p]
        ztp = np.ascontiguousarray(
            zt.reshape(B_LOC, KBP, 2, 128, L).transpose(0, 1, 3, 2, 4)
        ).astype(fp8)
        in_maps.append({"zt": ztp, "wt": wt})
    return in_maps


def _host_finish(results, tags, start_t, end_t, bias_c, transitions):
    """Combine per-core em outputs into the scalar loss (float64 host math)."""
    st = start_t.astype(np.float64)
    en = end_t.astype(np.float64)
    cb = bias_c.astype(np.float64)
    tr = transitions.astype(np.float64)

    em_all = np.concatenate(
        [np.asarray(results[ci]["em_out"], dtype=np.float64)
         for ci in range(N_CORES)], axis=0
    ) / WSCALE  # [B, C, L]

    tags = tags.astype(np.int64)
    l_idx = np.arange(L)
    b_idx = np.arange(B)[:, None]

    # numerator
    em_tag_sum = em_all[b_idx, tags, l_idx[None, :]].sum(axis=1)  # [B]
    bias_sum = cb[tags].sum(axis=1)
    trans_sum = tr[tags[:, :-1], tags[:, 1:]].sum(axis=1)
    numerator = st[tags[:, 0]] + en[tags[:, -1]] + em_tag_sum + bias_sum + trans_sum

    # log_z: forward recurrence in probability space with renormalization.
    # alpha_{t} = (alpha_{t-1} @ EB) * exp(em_t - m_t)
    EB = np.exp(tr + cb[None, :])  # [C, C] includes per-step bias
    alpha0 = st + cb + em_all[:, :, 0].copy()  # [B, C] log space
    m0 = alpha0.max(axis=1)
    v = np.exp(alpha0 - m0[:, None])
    log_z = m0.copy()
    for t in range(1, L):
        e_t = em_all[:, :, t]  # [B, C]
        m_t = e_t.max(axis=1)
        v = (v @ EB) * np.exp(e_t - m_t[:, None])
        s = v.max(axis=1)
        v /= s[:, None]
        log_z += np.log(s) + m_t
    log_z += np.log((v * np.exp(en)[None, :]).sum(axis=1))

    return np.float32(np.mean(log_z - numerator))


def kernel(**inputs):
    from concourse.bass_utils import run_bass_kernel_spmd

    Z = np.asarray(inputs["Z"], dtype=np.float32)
    tags = np.asarray(inputs["tags"])
    W = np.asarray(inputs["W"], dtype=np.float32)
    b_ = np.asarray(inputs["b"], dtype=np.float32)
    cb = np.asarray(inputs["class_bias"], dtype=np.float32)
    st = np.asarray(inputs["start_trans"], dtype=np.float32)
    en = np.asarray(inputs["end_trans"], dtype=np.float32)
    tr = np.asarray(inputs["transitions"], dtype=np.float32)

    bias_c = b_ + cb
    nc = _get_nc()
    in_maps = _host_prep(Z, W, bias_c, tr)
    res = run_bass_kernel_spmd(nc, in_maps, core_ids=list(range(N_CORES)))
    return _host_finish(res.results, tags, st, en, bias_c, tr)
